# revision 5
# baseline (speedup 1.0000x reference)
"""VQ codebook-lookup (kmeans assign) kernel for 8 Trainium2 NeuronCores.

Math: reference computes, per row n of xf = x.reshape(-1, D):
    xn = xf / max(||xf||, 1e-12)
    d[n,s] = ||xn||^2 + d2[s] - 2 * xn . c_s       (d2[s] = ||c_s||^2)
    inds[n] = argmin_s d[n,s];  quantized = codebook[:, inds]

||xn||^2 is constant per row, so argmin_s d = argmin_s (d2[s] - 2*xn.c_s).
Multiplying by the positive constant R_n/2 (R_n = max(||xf_n||,1e-12)):
    argmin_s d = argmax_s (xf_n . c_s - (R_n/2) * d2[s])
which is a single augmented matmul: lhs rows [xf_n, R_n/2], rhs columns
[c_s; -d2[s]].  So the kernel never normalizes x; it computes R_n on device
(square + ones-matmul reduction), runs the K=1281 augmented matmul in fp32,
takes a running per-512-chunk max8/max_index, resolves the global argmax,
gathers codebook rows via indirect DMA and transposes them back with the PE.

Sharding: data-parallel over the flattened N=B*T dim; core b handles batch b
(2048 rows).  The codebook (+augmented row) is replicated on every core.
"""

import math
import numpy as np

P = 128
N_CORES = 8

_NC_CACHE = {}


def _build_nc(NT, SC, KT, mm_dtype_name="float32"):
    """Build the per-core Bass program.

    NT: number of 128-row n-tiles (rows of x handled by this core = NT*128)
    SC: number of 512-wide codebook chunks (S = SC*512)
    KT: number of 128-deep contraction tiles (D = KT*128)
    """
    from contextlib import ExitStack

    import concourse.bass as bass
    import concourse.bacc as bacc
    import concourse.mybir as mybir
    import concourse.tile as tile
    from concourse.masks import make_identity

    f32 = mybir.dt.float32
    u32 = mybir.dt.uint32
    i32 = mybir.dt.int32
    mmdt = getattr(mybir.dt, mm_dtype_name)

    N = NT * P
    S = SC * 512
    D = KT * P
    NH = (NT + 7) // 8  # n-halves: groups of <=8 n-tiles (8 PSUM banks)

    nc = bacc.Bacc()
    xT_d = nc.dram_tensor("xT", [D, N], f32, kind="ExternalInput")
    cb_d = nc.dram_tensor("cb", [D + 1, S], f32, kind="ExternalInput")
    cbT_d = nc.dram_tensor("cbT", [S, D], f32, kind="ExternalInput")
    inds_d = nc.dram_tensor("inds", [N, 1], i32, kind="ExternalOutput")
    qT_d = nc.dram_tensor("qT", [D, N], f32, kind="ExternalOutput")

    with tile.TileContext(nc) as tc, ExitStack() as ctx:
        xpool = ctx.enter_context(tc.tile_pool(name="x", bufs=1))
        cbpool = ctx.enter_context(tc.tile_pool(name="cb", bufs=2))
        pspool = ctx.enter_context(tc.tile_pool(name="ps", bufs=8, space="PSUM"))
        spool = ctx.enter_context(tc.tile_pool(name="small", bufs=4))
        accpool = ctx.enter_context(tc.tile_pool(name="acc", bufs=2))
        gpool = ctx.enter_context(tc.tile_pool(name="gath", bufs=2))
        qpool = ctx.enter_context(tc.tile_pool(name="qout", bufs=4))
        cpool = ctx.enter_context(tc.tile_pool(name="const", bufs=1))
        sqpool = ctx.enter_context(tc.tile_pool(name="sq", bufs=2))

        # --- constants -----------------------------------------------------
        ident = cpool.tile([P, P], f32, tag="ident")
        make_identity(nc, ident[:])
        ones = cpool.tile([P, 1], f32, tag="ones")
        nc.vector.memset(ones[:], 1.0)
        iota = cpool.tile([P, SC], f32, tag="iota")
        for c in range(SC):
            nc.vector.memset(iota[:, c : c + 1], float(c))

        # --- load xT (stationary for the whole kernel) ---------------------
        xtiles = []
        for k in range(KT):
            t = xpool.tile([P, N], f32, tag=f"x{k}")
            nc.sync.dma_start(t[:], xT_d[k * P : (k + 1) * P, :])
            xtiles.append(t)

        # --- row norms: r2[n] = sum_d x[d,n]^2 via squares + ones-matmul ---
        nj_slices = [slice(j, min(j + 512, N)) for j in range(0, N, 512)]
        r2ps = [
            pspool.tile([1, sl.stop - sl.start], f32, tag="ps", name=f"r2ps{j}")
            for j, sl in enumerate(nj_slices)
        ]
        for k in range(KT):
            sq = sqpool.tile([P, N], f32, tag="sq")
            nc.vector.tensor_tensor(
                out=sq[:], in0=xtiles[k][:], in1=xtiles[k][:], op=mybir.AluOpType.mult
            )
            for j, sl in enumerate(nj_slices):
                nc.tensor.matmul(
                    out=r2ps[j][:],
                    lhsT=ones[:],
                    rhs=sq[:, sl],
                    start=(k == 0),
                    stop=(k == KT - 1),
                )
        # aug row = 0.5 * max(||x_n||, 1e-12) = sqrt(0.25 * max(r2, 1e-24))
        r2sb = cpool.tile([1, N], f32, tag="r2")
        for j, sl in enumerate(nj_slices):
            nc.vector.tensor_scalar_max(r2sb[:, sl], r2ps[j][:], 1e-24)
        augrow = cpool.tile([1, N], f32, tag="aug")
        nc.scalar.activation(
            augrow[:], r2sb[:], mybir.ActivationFunctionType.Sqrt, scale=0.25
        )

        # --- main loop ------------------------------------------------------
        for nh in range(NH):
            nts = [nt for nt in range(nh * 8, min((nh + 1) * 8, NT))]
            vacc = {nt: accpool.tile([P, SC], f32, tag=f"va{nt % 8}", name=f"va{nt}") for nt in nts}
            iacc = {nt: accpool.tile([P, SC], f32, tag=f"ia{nt % 8}", name=f"ia{nt}") for nt in nts}
            for sc in range(SC):
                cbt = []
                for k in range(KT):
                    t = cbpool.tile([P, 512], f32, tag=f"cb{k}")
                    nc.sync.dma_start(
                        t[:], cb_d[k * P : (k + 1) * P, sc * 512 : (sc + 1) * 512]
                    )
                    cbt.append(t)
                d2t = cbpool.tile([1, 512], f32, tag="d2", name=f"d2_{nh}_{sc}")
                nc.sync.dma_start(d2t[:], cb_d[D : D + 1, sc * 512 : (sc + 1) * 512])
                ps = {nt: pspool.tile([P, 512], f32, tag="ps", name=f"ps{nt}") for nt in nts}
                for k in range(KT):
                    for nt in nts:
                        nc.tensor.matmul(
                            out=ps[nt][:],
                            lhsT=xtiles[k][:, nt * P : (nt + 1) * P],
                            rhs=cbt[k][:],
                            start=(k == 0),
                            stop=False,
                        )
                for nt in nts:
                    # augmented K=1 tile folds in the -(R/2)*d2[s] term
                    nc.tensor.matmul(
                        out=ps[nt][:],
                        lhsT=augrow[:1, nt * P : (nt + 1) * P],
                        rhs=d2t[:1, :],
                        start=False,
                        stop=True,
                    )
                for nt in nts:
                    mv = spool.tile([P, 8], f32, tag="mv")
                    nc.vector.max(out=mv[:], in_=ps[nt][:])
                    mi = spool.tile([P, 8], u32, tag="mi")
                    nc.vector.max_index(out=mi[:], in_max=mv[:], in_values=ps[nt][:])
                    nc.scalar.copy(vacc[nt][:, sc : sc + 1], mv[:, 0:1])
                    mif = spool.tile([P, 1], f32, tag="mif")
                    nc.vector.tensor_copy(mif[:], mi[:, 0:1])
                    nc.vector.tensor_scalar_add(
                        iacc[nt][:, sc : sc + 1], mif[:], float(sc * 512)
                    )
            # resolve global argmax per n-tile; gather + transpose + store
            for nt in nts:
                fv = spool.tile([P, 8], f32, tag="fv")
                nc.vector.max(out=fv[:], in_=vacc[nt][:])
                fi = spool.tile([P, 8], u32, tag="fi")
                nc.vector.max_index(out=fi[:], in_max=fv[:], in_values=vacc[nt][:])
                fif = spool.tile([P, 1], f32, tag="fif")
                nc.vector.tensor_copy(fif[:], fi[:, 0:1])
                oh = spool.tile([P, SC], f32, tag="oh")
                nc.vector.tensor_tensor(
                    out=oh[:],
                    in0=iota[:],
                    in1=fif[:].to_broadcast([P, SC]),
                    op=mybir.AluOpType.is_equal,
                )
                sel = spool.tile([P, SC], f32, tag="sel")
                nc.vector.tensor_tensor(
                    out=sel[:], in0=oh[:], in1=iacc[nt][:], op=mybir.AluOpType.mult
                )
                gidxf = spool.tile([P, 1], f32, tag="gidxf")
                nc.vector.reduce_sum(out=gidxf[:], in_=sel[:], axis=mybir.AxisListType.X)
                indsi = spool.tile([P, 1], i32, tag="indsi")
                nc.vector.tensor_copy(indsi[:], gidxf[:])
                nc.sync.dma_start(inds_d[nt * P : (nt + 1) * P, :], indsi[:])
                gidx = spool.tile([P, 1], u32, tag="gidx")
                nc.vector.tensor_copy(gidx[:], gidxf[:])
                gath = gpool.tile([P, D], f32, tag="gath")
                nc.gpsimd.indirect_dma_start(
                    out=gath[:],
                    out_offset=None,
                    in_=cbT_d[:],
                    in_offset=bass.IndirectOffsetOnAxis(ap=gidx[:, :1], axis=0),
                )
                for db in range(KT):
                    pt = pspool.tile([P, P], f32, tag="ps")
                    nc.tensor.transpose(
                        pt[:], gath[:, db * P : (db + 1) * P], ident[:]
                    )
                    qst = qpool.tile([P, P], f32, tag="qst")
                    nc.scalar.copy(qst[:], pt[:])
                    nc.sync.dma_start(
                        qT_d[db * P : (db + 1) * P, nt * P : (nt + 1) * P], qst[:]
                    )
    nc.compile()
    return nc


def _get_nc(NT=16, SC=16, KT=10, mm_dtype_name="float32"):
    key = (NT, SC, KT, mm_dtype_name)
    if key not in _NC_CACHE:
        _NC_CACHE[key] = _build_nc(NT, SC, KT, mm_dtype_name)
    return _NC_CACHE[key]


def kernel(x: np.ndarray, codebook: np.ndarray):
    from concourse.bass_utils import run_bass_kernel_spmd

    x = np.asarray(x, dtype=np.float32)
    codebook = np.asarray(codebook, dtype=np.float32)
    B, T, D = x.shape
    S = codebook.shape[1]
    assert B == N_CORES and D == codebook.shape[0]

    d2 = (codebook * codebook).sum(axis=0, dtype=np.float32)
    cb_aug = np.concatenate([codebook, -d2[None, :]], axis=0)  # [D+1, S]
    cbT = np.ascontiguousarray(codebook.T)  # [S, D]

    nc = _get_nc(NT=T // P, SC=S // 512, KT=D // P)
    in_maps = [
        {
            "xT": np.ascontiguousarray(x[c].T),  # [D, T]
            "cb": cb_aug,
            "cbT": cbT,
        }
        for c in range(N_CORES)
    ]
    res = run_bass_kernel_spmd(nc, in_maps, list(range(N_CORES))).results

    inds = np.stack(
        [res[c]["inds"].reshape(T).astype(np.int32) for c in range(N_CORES)]
    )  # [B, T]
    quantized = np.stack([res[c]["qT"] for c in range(N_CORES)], axis=1)  # [D, B, T]
    return inds, quantized
